# revision 22
# baseline (speedup 1.0000x reference)
# Trainium2 Bass kernel for the Chebyshev-GCN GRU decoder (gnn_message_passing).
#
# Problem: B=16, N=2048, F=64, K=2 Chebyshev taps, T=8 decode steps.
#   per step: gates = cheb(L, [x, hx]) @ W_gate; r,u = sigmoid(gates)
#             cy = tanh(cheb(L, [x, r*hx]) @ W_upd); hy = u*hx + (1-u)*cy
#             yt = sigmoid(hy @ W_edge)
#
# Strategy (all math on device; host does only layout transforms + sharding):
#  - Data-parallel over batch: 8 cores x 2 batches each.
#  - x is re-fed every step, so all x-only terms are step-invariant:
#      L@x, x@W*_x, (L@x)@W*_x  ->  computed once on device ("Gconst"/"Cconst").
#    The per-step big matmuls shrink to L@hx and L@(r*hx)  (N x N x 64 each).
#  - Everything lives in "transposed" layout [c, n] with c = b*64 + f (128
#    partitions = 2 batches x 64 features), so the small (feature) matmuls
#    contract over partitions.  The two batches are kept independent in one
#    128-wide matmul by block-diagonal 128x128 weights (built on host).
#  - Big matmul orientation: out[c, n] += sum_m hx_nat[m, c] * L^T[m, n]:
#    stationary = hx in natural layout (16 tiles of [128m, 128c]), moving =
#    L^T (free dim 512 per instruction).  L^T stays SBUF-resident (14 of 16
#    row-blocks; 2 streamed from HBM per use to fit SBUF).
#  - hx/r*hx needed both transposed (elementwise/small-mm) and natural
#    (stationary): regenerated each step with PE-mode transposes (16+16 tiles).
#  - Matmul dtype is a knob: float32r (single-pass fp32, full PE rate at free
#    dim >= 256) vs float32 (exact, 4 cycles/row).  PSUM accumulation is fp32
#    either way.
#
# The kernel() entry point takes FULL unsharded inputs and returns the FULL
# [T, B, N, F] output; it shards/reassembles on host.

import numpy as np
from contextlib import ExitStack

import concourse.bass as bass
import concourse.tile as tile
from concourse import bacc, mybir
from concourse.bass_utils import run_bass_kernel_spmd

F32 = mybir.dt.float32

B, N, F = 16, 2048, 64
T = 8
NCORES = 8
BL = B // NCORES          # batches per core (2)
C = BL * F                # 128 = partition width of transposed tensors
NT = N // 128             # 16 contraction tiles
NBLK = 4                  # n blocks per big matmul
BLK = N // NBLK           # 512 = free dim per matmul instruction
LT_RES = 16               # L^T row-blocks resident in SBUF (all)

# Matmul-operand dtype knob.  float32r = single-pass fp32 matmul (full PE
# rate at free dim >= 256, reduced multiply precision); float32 = exact,
# 4 cycles/row.  walrus requires fp32r operands to be *produced* as fp32r,
# so every tensor feeding a matmul is declared MM_DT end-to-end (same bytes
# as fp32 in memory; numpy side stays float32).
MM_DT = mybir.dt.float32r
BF16 = mybir.dt.bfloat16   # storage dtype of the step-invariant gate consts

W_NAMES = [
    "wh0r", "wh0u", "wh1r", "wh1u",   # gates, hx / L@hx terms (r and u halves)
    "wx0r", "wx0u", "wx1r", "wx1u",   # gates, x / L@x terms (precompute)
    "whc0", "whc1",                   # candidate, r*hx / L@(r*hx) terms
    "wxc0", "wxc1",                   # candidate, x / L@x terms (precompute)
    "we",                             # edge output projection
]
B_NAMES = ["bgr", "bgu", "bcc", "bee"]


def _emit(ctx: ExitStack, tc: tile.TileContext, d):
    """Emit the per-core program.  `d` maps dram tensor name -> AP."""
    nc = tc.nc
    AF = mybir.ActivationFunctionType

    consts = ctx.enter_context(tc.tile_pool(name="consts", bufs=1))
    work = ctx.enter_context(tc.tile_pool(name="work", bufs=2))
    tmp3 = ctx.enter_context(tc.tile_pool(name="tmp", bufs=3))
    big_ps = ctx.enter_context(tc.tile_pool(name="bigps", bufs=2, space="PSUM"))
    small_ps = ctx.enter_context(tc.tile_pool(name="smallps", bufs=4, space="PSUM"))
    tr_ps = ctx.enter_context(tc.tile_pool(name="trps", bufs=2, space="PSUM"))

    # ---- static loads -------------------------------------------------
    # all 13 weights + identity packed into one DMA; biases in another
    wpack = consts.tile([128, 14 * 128], MM_DT, tag="wpack")
    nc.sync.dma_start(wpack[:], d["wpack"][:, :])
    w = {name: wpack[:, i * 128:(i + 1) * 128]
         for i, name in enumerate(W_NAMES)}
    ident = wpack[:, 13 * 128:14 * 128]
    bpack = consts.tile([128, len(B_NAMES)], F32, tag="bpack")
    nc.sync.dma_start(bpack[:], d["bpack"][:, :])
    bias = {name: bpack[:, j:j + 1] for j, name in enumerate(B_NAMES)}

    # stationary buffer: holds x_nat, then alternates hx_nat -> rh_nat per step
    s_sb = consts.tile([128, N], MM_DT, tag="s")
    for mi in range(NT):
        nc.sync.dma_start(s_sb[:, mi * 128:(mi + 1) * 128],
                          d["xnat"][mi * 128:(mi + 1) * 128, :])
    # xt shares its slot with uT (dead after precompute)
    xt_sb = consts.tile([128, N], MM_DT, tag="xt_u")
    nc.sync.dma_start(xt_sb[:], d["xt"][:, :])

    # L^T load is the long pole at kernel start (~14MB): emitted LAST among
    # the static loads (the dynamic-DGE ring issues in order, so anything
    # after it would stall ~40us) and as 256KB chunks in the order the
    # first big matmuls consume them, so PE starts once chunk (0,0) lands.
    lt_sb = consts.tile([128, LT_RES * N], MM_DT, tag="ltsb")
    for blk in range(NBLK):
        for mi in range(LT_RES):
            nc.sync.dma_start(
                lt_sb[:, mi * N + blk * BLK: mi * N + (blk + 1) * BLK],
                d["lt"][mi * 128:(mi + 1) * 128, blk * BLK:(blk + 1) * BLK])

    hxbuf = [consts.tile([128, N], MM_DT, tag=f"hxT{i}", name=f"hxT{i}")
             for i in range(2)]
    rhT = consts.tile([128, N], MM_DT, tag="rhT")
    grc = consts.tile([128, N], BF16, tag="grc")
    guc = consts.tile([128, N], BF16, tag="guc")
    ccc = consts.tile([128, N], BF16, tag="ccc")
    # full-width landing buffer for (L @ stationary)^T; shared by the
    # precompute LxT, phase-A LhT and phase-B LrhT (disjoint lifetimes)
    lxh = consts.tile([128, N], MM_DT, tag="lxh")

    def lt_rhs(mi, blk):
        """[128, BLK] moving-operand slice of L^T for row-block mi, n-block blk."""
        return lt_sb[:, mi * N + blk * BLK: mi * N + (blk + 1) * BLK]

    def big_mm(blk):
        """psum[c, n_blk] = sum_m s_sb[m, c] * L^T[m, n_blk]  (16-tile accum)."""
        ps = big_ps.tile([128, BLK], F32, tag="big")
        for mi in range(NT):
            nc.tensor.matmul(
                ps[:],
                s_sb[:, mi * 128:(mi + 1) * 128],
                lt_rhs(mi, blk),
                start=(mi == 0), stop=(mi == NT - 1))
        return ps

    def small_mm(pairs, const_ap=None):
        """psum = sum_i w_i.T @ rhs_i; then += const_ap in place (DVE)."""
        ps = small_ps.tile([128, BLK], F32, tag="small")
        for i, (wt, rhs) in enumerate(pairs):
            nc.tensor.matmul(ps[:], wt[:], rhs,
                             start=(i == 0), stop=(i == len(pairs) - 1))
        if const_ap is not None:
            nc.vector.tensor_add(ps[:], ps[:], const_ap)
        return ps

    def transpose_to_s(src, blk):
        """PE-transpose 4 [128,128] tiles of src n-block blk into s_sb."""
        pt = tr_ps.tile([128, BLK], MM_DT, tag="tr")
        for j in range(4):
            mi = blk * 4 + j
            nc.tensor.transpose(pt[:, j * 128:(j + 1) * 128],
                                src[:, mi * 128:(mi + 1) * 128], ident[:])
        nc.vector.tensor_copy(s_sb[:, blk * BLK:(blk + 1) * BLK], pt[:])

    def nb(ap, blk):
        return ap[:, blk * BLK:(blk + 1) * BLK]

    # ---- precompute: LxT = (L@x)^T, then the step-invariant gate/cand consts
    for blk in range(NBLK):
        ps = big_mm(blk)                       # s_sb holds x_nat here
        nc.vector.tensor_copy(nb(lxh, blk), ps[:])
    for blk in range(NBLK):
        for wa, wb, bi, dst in (("wx0r", "wx1r", "bgr", grc),
                                ("wx0u", "wx1u", "bgu", guc),
                                ("wxc0", "wxc1", "bcc", ccc)):
            psg = small_mm([(w[wa], nb(xt_sb, blk)), (w[wb], nb(lxh, blk))])
            nc.scalar.activation(nb(dst, blk), psg[:], AF.Identity, bias=bias[bi][:])

    uT = consts.tile([128, N], F32, tag="xt_u")   # reuses xt slot

    def emit_out(t, hyT, blk):
        """yt = sigmoid(W_edge.T @ hy + b_edge) -> DRAM out[t]."""
        ps = small_mm([(w["we"], nb(hyT, blk))])
        ytt = tmp3.tile([128, BLK], F32, tag="tmp")
        nc.scalar.activation(ytt[:], ps[:], AF.Sigmoid, bias=bias["bee"][:])
        nc.sync.dma_start(d["out"][t, :, blk * BLK:(blk + 1) * BLK], ytt[:])

    # ---- step 0 (hx == 0: no big matmuls, r unused) -------------------
    hyT = hxbuf[1]
    for blk in range(NBLK):
        nc.scalar.activation(nb(uT, blk), nb(guc, blk), AF.Sigmoid)
        cyt = work.tile([128, BLK], F32, tag="cyt")
        nc.scalar.activation(cyt[:], nb(ccc, blk), AF.Tanh)
        e = tmp3.tile([128, BLK], F32, tag="tmp")
        nc.vector.tensor_mul(e[:], nb(uT, blk), cyt[:])
        nc.vector.tensor_sub(nb(hyT, blk), cyt[:], e[:])   # hy0 = (1-u)*cy
        emit_out(0, hyT, blk)
        transpose_to_s(hyT, blk)

    # ---- steps 1..T-1 -------------------------------------------------
    for t in range(1, T):
        hxT, hyT = hxbuf[t % 2], hxbuf[(t + 1) % 2]
        # phase A1: Lh = (L@hx)^T for ALL blocks (s_sb must stay hx_nat
        # until every big matmul has read it)
        for blk in range(NBLK):
            ps = big_mm(blk)                   # s_sb holds hx_nat
            nc.vector.tensor_copy(nb(lxh, blk), ps[:])
        # phase A2: r,u; rh = r*hx; transpose rh -> s_sb.
        # Emission order = engine-queue order, so everything on the
        # r-critical path (r matmuls, r const-adds, r sigmoids, rh muls,
        # transposes) is emitted before the u-gate work, which is only
        # needed late in phase B2.
        psrs = [small_mm([(w["wh0r"], nb(hxT, blk)),
                          (w["wh1r"], nb(lxh, blk))],
                         const_ap=nb(grc, blk)) for blk in range(NBLK)]
        psus = [small_mm([(w["wh0u"], nb(hxT, blk)),
                          (w["wh1u"], nb(lxh, blk))]) for blk in range(NBLK)]
        for blk in range(NBLK):
            nc.scalar.activation(nb(rhT, blk), psrs[blk][:], AF.Sigmoid)
        for blk in range(NBLK):
            nc.vector.tensor_mul(nb(rhT, blk), nb(rhT, blk), nb(hxT, blk))
        for blk in range(NBLK):
            transpose_to_s(rhT, blk)
        for blk in range(NBLK):
            nc.vector.tensor_add(psus[blk][:], psus[blk][:], nb(guc, blk))
            nc.scalar.activation(nb(uT, blk), psus[blk][:], AF.Sigmoid)
        # phase B1: Lrh = (L@(r*hx))^T for ALL blocks
        for blk in range(NBLK):
            ps = big_mm(blk)                   # s_sb holds rh_nat
            nc.scalar.copy(nb(lxh, blk), ps[:])
        # phase B2: cy; hy; yt; transpose hy -> s_sb (same split as A2)
        pscs = []
        for blk in range(NBLK):
            pscs.append(small_mm([(w["whc0"], nb(rhT, blk)),
                                  (w["whc1"], nb(lxh, blk))],
                                 const_ap=nb(ccc, blk)))
        for blk in range(NBLK):
            cyt = work.tile([128, BLK], F32, tag="cyt")
            nc.scalar.activation(cyt[:], pscs[blk][:], AF.Tanh)
            dd = tmp3.tile([128, BLK], F32, tag="tmp")
            nc.vector.tensor_sub(dd[:], nb(hxT, blk), cyt[:])
            ee = tmp3.tile([128, BLK], F32, tag="tmp")
            nc.vector.tensor_mul(ee[:], nb(uT, blk), dd[:])
            nc.vector.tensor_add(nb(hyT, blk), cyt[:], ee[:])  # hy = cy + u*(hx-cy)
            emit_out(t, hyT, blk)
            if t < T - 1:
                transpose_to_s(hyT, blk)


_BUILT = {}


def _build():
    if "nc" in _BUILT:
        return _BUILT["nc"]
    nc = bacc.Bacc("TRN2", target_bir_lowering=False, debug=False)
    d = {}
    d["lt"] = nc.dram_tensor("lt", [N, N], MM_DT, kind="ExternalInput").ap()
    d["xnat"] = nc.dram_tensor("xnat", [N, C], MM_DT, kind="ExternalInput").ap()
    d["xt"] = nc.dram_tensor("xt", [C, N], MM_DT, kind="ExternalInput").ap()
    d["wpack"] = nc.dram_tensor("wpack", [128, 14 * 128], MM_DT,
                                kind="ExternalInput").ap()
    d["bpack"] = nc.dram_tensor("bpack", [128, len(B_NAMES)], F32,
                                kind="ExternalInput").ap()
    d["out"] = nc.dram_tensor("out", [T, C, N], F32, kind="ExternalOutput").ap()

    with tile.TileContext(nc) as tc, ExitStack() as ctx:
        _emit(ctx, tc, d)
    nc.compile()
    _BUILT["nc"] = nc
    return nc


def _bd(m):
    """[64,64] -> block-diagonal [128,128] (two independent batches)."""
    z = np.zeros((128, 128), np.float32)
    z[:64, :64] = m
    z[64:, 64:] = m
    return z


def make_in_maps(inputs_edge, L_tilde, W_gate, b_gate, W_upd, b_upd,
                 W_edge, b_edge):
    """Host-side layout transforms + per-core sharding (no math)."""
    x = np.asarray(inputs_edge, np.float32)
    L = np.asarray(L_tilde, np.float32)
    Wg0, Wg1 = np.asarray(W_gate[0], np.float32), np.asarray(W_gate[1], np.float32)
    Wu0, Wu1 = np.asarray(W_upd[0], np.float32), np.asarray(W_upd[1], np.float32)
    We = np.asarray(W_edge, np.float32)
    bg = np.asarray(b_gate, np.float32)
    bu = np.asarray(b_upd, np.float32)
    be = np.asarray(b_edge, np.float32)

    wmats = {
        "wh0r": _bd(Wg0[64:, :64]), "wh0u": _bd(Wg0[64:, 64:]),
        "wh1r": _bd(Wg1[64:, :64]), "wh1u": _bd(Wg1[64:, 64:]),
        "wx0r": _bd(Wg0[:64, :64]), "wx0u": _bd(Wg0[:64, 64:]),
        "wx1r": _bd(Wg1[:64, :64]), "wx1u": _bd(Wg1[:64, 64:]),
        "whc0": _bd(Wu0[64:]), "whc1": _bd(Wu1[64:]),
        "wxc0": _bd(Wu0[:64]), "wxc1": _bd(Wu1[:64]),
        "we": _bd(We),
    }
    wpack = np.concatenate([wmats[n] for n in W_NAMES]
                           + [np.eye(128, dtype=np.float32)], axis=1)
    bpack = np.stack([np.tile(bg[:64], 2), np.tile(bg[64:], 2),
                      np.tile(bu, 2), np.tile(be, 2)], axis=1)
    shared = {
        "lt": np.ascontiguousarray(L.T),
        "wpack": np.ascontiguousarray(wpack),
        "bpack": np.ascontiguousarray(bpack.astype(np.float32)),
    }
    in_maps = []
    for core in range(NCORES):
        xs = x[core * BL:(core + 1) * BL]                    # [BL, N, F]
        m = dict(shared)
        m["xnat"] = np.ascontiguousarray(xs.transpose(1, 0, 2).reshape(N, C))
        m["xt"] = np.ascontiguousarray(xs.transpose(0, 2, 1).reshape(C, N))
        in_maps.append(m)
    return in_maps


def unshard(core_outs):
    """[NCORES][T, C, N] -> [T, B, N, F]"""
    arr = np.stack(core_outs)                                # [8, T, 128, N]
    return np.ascontiguousarray(
        arr.reshape(NCORES, T, BL, F, N)
           .transpose(1, 0, 2, 4, 3)
           .reshape(T, B, N, F).astype(np.float32))


def run(in_maps, **kw):
    nc = _build()
    return run_bass_kernel_spmd(nc, in_maps, list(range(NCORES)), **kw)


def kernel(inputs_edge, L_tilde, W_gate, b_gate, W_upd, b_upd, W_edge, b_edge):
    in_maps = make_in_maps(inputs_edge, L_tilde, W_gate, b_gate,
                           W_upd, b_upd, W_edge, b_edge)
    res = run(in_maps)
    return unshard([res.results[c]["out"] for c in range(NCORES)])


# revision 23
# speedup vs baseline: 13767.0269x; 13767.0269x over previous
# Trainium2 Bass kernel for the Chebyshev-GCN GRU decoder (gnn_message_passing).
#
# Problem: B=16, N=2048, F=64, K=2 Chebyshev taps, T=8 decode steps.
#   per step: gates = cheb(L, [x, hx]) @ W_gate; r,u = sigmoid(gates)
#             cy = tanh(cheb(L, [x, r*hx]) @ W_upd); hy = u*hx + (1-u)*cy
#             yt = sigmoid(hy @ W_edge)
#
# Strategy (all math on device; host does only layout transforms + sharding):
#  - Data-parallel over batch: 8 cores x 2 batches each.
#  - x is re-fed every step, so all x-only terms are step-invariant:
#      L@x, x@W*_x, (L@x)@W*_x  ->  computed once on device ("Gconst"/"Cconst").
#    The per-step big matmuls shrink to L@hx and L@(r*hx)  (N x N x 64 each).
#  - Everything lives in "transposed" layout [c, n] with c = b*64 + f (128
#    partitions = 2 batches x 64 features), so the small (feature) matmuls
#    contract over partitions.  The two batches are kept independent in one
#    128-wide matmul by block-diagonal 128x128 weights (built on host).
#  - Big matmul orientation: out[c, n] += sum_m hx_nat[m, c] * L^T[m, n]:
#    stationary = hx in natural layout (16 tiles of [128m, 128c]), moving =
#    L^T (free dim 512 per instruction).  L^T stays fully SBUF-resident
#    (fits because the step-invariant gate consts are stored bf16).
#  - hx/r*hx needed both transposed (elementwise/small-mm) and natural
#    (stationary): regenerated each step with PE-mode transposes (16+16 tiles).
#  - Matmul dtype is a knob: float32r (single-pass fp32, full PE rate at free
#    dim >= 256) vs float32 (exact, 4 cycles/row).  PSUM accumulation is fp32
#    either way.
#
# The kernel() entry point takes FULL unsharded inputs and returns the FULL
# [T, B, N, F] output; it shards/reassembles on host.

import numpy as np
from contextlib import ExitStack

import concourse.bass as bass
import concourse.tile as tile
from concourse import bacc, mybir
from concourse.bass_utils import run_bass_kernel_spmd

F32 = mybir.dt.float32

B, N, F = 16, 2048, 64
T = 8
NCORES = 8
BL = B // NCORES          # batches per core (2)
C = BL * F                # 128 = partition width of transposed tensors
NT = N // 128             # 16 contraction tiles
NBLK = 4                  # n blocks per big matmul
BLK = N // NBLK           # 512 = free dim per matmul instruction
LT_RES = NT               # all 16 L^T row-blocks stay SBUF-resident

# Matmul-operand dtype knob.  float32r = single-pass fp32 matmul (full PE
# rate at free dim >= 256, reduced multiply precision); float32 = exact,
# 4 cycles/row.  walrus requires fp32r operands to be *produced* as fp32r,
# so every tensor feeding a matmul is declared MM_DT end-to-end (same bytes
# as fp32 in memory; numpy side stays float32).
MM_DT = mybir.dt.float32r
BF16 = mybir.dt.bfloat16   # storage dtype of the step-invariant gate consts

W_NAMES = [
    "wh0r", "wh0u", "wh1r", "wh1u",   # gates, hx / L@hx terms (r and u halves)
    "wx0r", "wx0u", "wx1r", "wx1u",   # gates, x / L@x terms (precompute)
    "whc0", "whc1",                   # candidate, r*hx / L@(r*hx) terms
    "wxc0", "wxc1",                   # candidate, x / L@x terms (precompute)
    "we",                             # edge output projection
]
B_NAMES = ["bgr", "bgu", "bcc", "bee"]


def _emit(ctx: ExitStack, tc: tile.TileContext, d):
    """Emit the per-core program.  `d` maps dram tensor name -> AP."""
    nc = tc.nc
    AF = mybir.ActivationFunctionType

    consts = ctx.enter_context(tc.tile_pool(name="consts", bufs=1))
    work = ctx.enter_context(tc.tile_pool(name="work", bufs=2))
    tmp3 = ctx.enter_context(tc.tile_pool(name="tmp", bufs=3))
    big_ps = ctx.enter_context(tc.tile_pool(name="bigps", bufs=2, space="PSUM"))
    small_ps = ctx.enter_context(tc.tile_pool(name="smallps", bufs=4, space="PSUM"))
    tr_ps = ctx.enter_context(tc.tile_pool(name="trps", bufs=2, space="PSUM"))

    # ---- static loads -------------------------------------------------
    # all 13 weights + identity packed into one DMA; biases in another
    wpack = consts.tile([128, 14 * 128], MM_DT, tag="wpack")
    nc.sync.dma_start(wpack[:], d["wpack"][:, :])
    w = {name: wpack[:, i * 128:(i + 1) * 128]
         for i, name in enumerate(W_NAMES)}
    ident = wpack[:, 13 * 128:14 * 128]
    bpack = consts.tile([128, len(B_NAMES)], F32, tag="bpack")
    nc.sync.dma_start(bpack[:], d["bpack"][:, :])
    bias = {name: bpack[:, j:j + 1] for j, name in enumerate(B_NAMES)}

    # stationary buffer: holds x_nat, then alternates hx_nat -> rh_nat per step
    s_sb = consts.tile([128, N], MM_DT, tag="s")
    for mi in range(NT):
        nc.sync.dma_start(s_sb[:, mi * 128:(mi + 1) * 128],
                          d["xnat"][mi * 128:(mi + 1) * 128, :])
    # xt shares its slot with uT (dead after precompute)
    xt_sb = consts.tile([128, N], MM_DT, tag="xt_u")
    nc.sync.dma_start(xt_sb[:], d["xt"][:, :])

    # L^T load is the long pole at kernel start (~14MB): emitted LAST among
    # the static loads (the dynamic-DGE ring issues in order, so anything
    # after it would stall ~40us) and as 256KB chunks in the order the
    # first big matmuls consume them, so PE starts once chunk (0,0) lands.
    lt_sb = consts.tile([128, LT_RES * N], MM_DT, tag="ltsb")
    for blk in range(NBLK):
        for mi in range(LT_RES):
            nc.sync.dma_start(
                lt_sb[:, mi * N + blk * BLK: mi * N + (blk + 1) * BLK],
                d["lt"][mi * 128:(mi + 1) * 128, blk * BLK:(blk + 1) * BLK])

    hxbuf = [consts.tile([128, N], MM_DT, tag=f"hxT{i}", name=f"hxT{i}")
             for i in range(2)]
    rhT = consts.tile([128, N], MM_DT, tag="rhT")
    grc = consts.tile([128, N], BF16, tag="grc")
    guc = consts.tile([128, N], BF16, tag="guc")
    ccc = consts.tile([128, N], BF16, tag="ccc")
    # full-width landing buffer for (L @ stationary)^T; shared by the
    # precompute LxT, phase-A LhT and phase-B LrhT (disjoint lifetimes)
    lxh = consts.tile([128, N], MM_DT, tag="lxh")

    def lt_rhs(mi, blk):
        """[128, BLK] moving-operand slice of L^T for row-block mi, n-block blk."""
        return lt_sb[:, mi * N + blk * BLK: mi * N + (blk + 1) * BLK]

    def big_mm(blk):
        """psum[c, n_blk] = sum_m s_sb[m, c] * L^T[m, n_blk]  (16-tile accum)."""
        ps = big_ps.tile([128, BLK], F32, tag="big")
        for mi in range(NT):
            nc.tensor.matmul(
                ps[:],
                s_sb[:, mi * 128:(mi + 1) * 128],
                lt_rhs(mi, blk),
                start=(mi == 0), stop=(mi == NT - 1))
        return ps

    def small_mm(pairs, const_ap=None):
        """psum = sum_i w_i.T @ rhs_i; then += const_ap in place (DVE)."""
        ps = small_ps.tile([128, BLK], F32, tag="small")
        for i, (wt, rhs) in enumerate(pairs):
            nc.tensor.matmul(ps[:], wt[:], rhs,
                             start=(i == 0), stop=(i == len(pairs) - 1))
        if const_ap is not None:
            nc.vector.tensor_add(ps[:], ps[:], const_ap)
        return ps

    def transpose_to_s(src, blk):
        """PE-transpose 4 [128,128] tiles of src n-block blk into s_sb."""
        pt = tr_ps.tile([128, BLK], MM_DT, tag="tr")
        for j in range(4):
            mi = blk * 4 + j
            nc.tensor.transpose(pt[:, j * 128:(j + 1) * 128],
                                src[:, mi * 128:(mi + 1) * 128], ident[:])
        nc.vector.tensor_copy(s_sb[:, blk * BLK:(blk + 1) * BLK], pt[:])

    def nb(ap, blk):
        return ap[:, blk * BLK:(blk + 1) * BLK]

    # ---- precompute: LxT = (L@x)^T, then the step-invariant gate/cand consts
    for blk in range(NBLK):
        ps = big_mm(blk)                       # s_sb holds x_nat here
        nc.vector.tensor_copy(nb(lxh, blk), ps[:])
    for blk in range(NBLK):
        for wa, wb, bi, dst in (("wx0r", "wx1r", "bgr", grc),
                                ("wx0u", "wx1u", "bgu", guc),
                                ("wxc0", "wxc1", "bcc", ccc)):
            psg = small_mm([(w[wa], nb(xt_sb, blk)), (w[wb], nb(lxh, blk))])
            nc.scalar.activation(nb(dst, blk), psg[:], AF.Identity, bias=bias[bi][:])

    uT = consts.tile([128, N], F32, tag="xt_u")   # reuses xt slot

    def emit_out(t, hyT, blk):
        """yt = sigmoid(W_edge.T @ hy + b_edge) -> DRAM out[t]."""
        ps = small_mm([(w["we"], nb(hyT, blk))])
        ytt = tmp3.tile([128, BLK], F32, tag="tmp")
        nc.scalar.activation(ytt[:], ps[:], AF.Sigmoid, bias=bias["bee"][:])
        nc.sync.dma_start(d["out"][t, :, blk * BLK:(blk + 1) * BLK], ytt[:])

    # ---- step 0 (hx == 0: no big matmuls, r unused) -------------------
    hyT = hxbuf[1]
    for blk in range(NBLK):
        nc.scalar.activation(nb(uT, blk), nb(guc, blk), AF.Sigmoid)
        cyt = work.tile([128, BLK], F32, tag="cyt")
        nc.scalar.activation(cyt[:], nb(ccc, blk), AF.Tanh)
        e = tmp3.tile([128, BLK], F32, tag="tmp")
        nc.vector.tensor_mul(e[:], nb(uT, blk), cyt[:])
        nc.vector.tensor_sub(nb(hyT, blk), cyt[:], e[:])   # hy0 = (1-u)*cy
        emit_out(0, hyT, blk)
        transpose_to_s(hyT, blk)

    # ---- steps 1..T-1 -------------------------------------------------
    for t in range(1, T):
        hxT, hyT = hxbuf[t % 2], hxbuf[(t + 1) % 2]
        # phase A1: Lh = (L@hx)^T for ALL blocks (s_sb must stay hx_nat
        # until every big matmul has read it)
        for blk in range(NBLK):
            ps = big_mm(blk)                   # s_sb holds hx_nat
            nc.vector.tensor_copy(nb(lxh, blk), ps[:])
        # phase A2: r,u; rh = r*hx; transpose rh -> s_sb.
        # Emission order = engine-queue order, so everything on the
        # r-critical path (r matmuls, r const-adds, r sigmoids, rh muls,
        # transposes) is emitted before the u-gate work, which is only
        # needed late in phase B2.
        psrs = [small_mm([(w["wh0r"], nb(hxT, blk)),
                          (w["wh1r"], nb(lxh, blk))],
                         const_ap=nb(grc, blk)) for blk in range(NBLK)]
        psus = [small_mm([(w["wh0u"], nb(hxT, blk)),
                          (w["wh1u"], nb(lxh, blk))]) for blk in range(NBLK)]
        for blk in range(NBLK):
            nc.scalar.activation(nb(rhT, blk), psrs[blk][:], AF.Sigmoid)
        for blk in range(NBLK):
            nc.vector.tensor_mul(nb(rhT, blk), nb(rhT, blk), nb(hxT, blk))
        for blk in range(NBLK):
            transpose_to_s(rhT, blk)
        for blk in range(NBLK):
            nc.vector.tensor_add(psus[blk][:], psus[blk][:], nb(guc, blk))
            nc.scalar.activation(nb(uT, blk), psus[blk][:], AF.Sigmoid)
        # phase B1: Lrh = (L@(r*hx))^T for ALL blocks
        for blk in range(NBLK):
            ps = big_mm(blk)                   # s_sb holds rh_nat
            nc.scalar.copy(nb(lxh, blk), ps[:])
        # phase B2: cy; hy; yt; transpose hy -> s_sb (same split as A2)
        pscs = []
        for blk in range(NBLK):
            pscs.append(small_mm([(w["whc0"], nb(rhT, blk)),
                                  (w["whc1"], nb(lxh, blk))],
                                 const_ap=nb(ccc, blk)))
        for blk in range(NBLK):
            cyt = work.tile([128, BLK], F32, tag="cyt")
            nc.scalar.activation(cyt[:], pscs[blk][:], AF.Tanh)
            dd = tmp3.tile([128, BLK], F32, tag="tmp")
            nc.vector.tensor_sub(dd[:], nb(hxT, blk), cyt[:])
            ee = tmp3.tile([128, BLK], F32, tag="tmp")
            nc.vector.tensor_mul(ee[:], nb(uT, blk), dd[:])
            nc.vector.tensor_add(nb(hyT, blk), cyt[:], ee[:])  # hy = cy + u*(hx-cy)
            emit_out(t, hyT, blk)
            if t < T - 1:
                transpose_to_s(hyT, blk)


_BUILT = {}


def _build():
    if "nc" in _BUILT:
        return _BUILT["nc"]
    nc = bacc.Bacc("TRN2", target_bir_lowering=False, debug=False)
    d = {}
    d["lt"] = nc.dram_tensor("lt", [N, N], MM_DT, kind="ExternalInput").ap()
    d["xnat"] = nc.dram_tensor("xnat", [N, C], MM_DT, kind="ExternalInput").ap()
    d["xt"] = nc.dram_tensor("xt", [C, N], MM_DT, kind="ExternalInput").ap()
    d["wpack"] = nc.dram_tensor("wpack", [128, 14 * 128], MM_DT,
                                kind="ExternalInput").ap()
    d["bpack"] = nc.dram_tensor("bpack", [128, len(B_NAMES)], F32,
                                kind="ExternalInput").ap()
    d["out"] = nc.dram_tensor("out", [T, C, N], F32, kind="ExternalOutput").ap()

    with tile.TileContext(nc) as tc, ExitStack() as ctx:
        _emit(ctx, tc, d)
    nc.compile()
    _BUILT["nc"] = nc
    return nc


def _bd(m):
    """[64,64] -> block-diagonal [128,128] (two independent batches)."""
    z = np.zeros((128, 128), np.float32)
    z[:64, :64] = m
    z[64:, 64:] = m
    return z


def make_in_maps(inputs_edge, L_tilde, W_gate, b_gate, W_upd, b_upd,
                 W_edge, b_edge):
    """Host-side layout transforms + per-core sharding (no math)."""
    x = np.asarray(inputs_edge, np.float32)
    L = np.asarray(L_tilde, np.float32)
    Wg0, Wg1 = np.asarray(W_gate[0], np.float32), np.asarray(W_gate[1], np.float32)
    Wu0, Wu1 = np.asarray(W_upd[0], np.float32), np.asarray(W_upd[1], np.float32)
    We = np.asarray(W_edge, np.float32)
    bg = np.asarray(b_gate, np.float32)
    bu = np.asarray(b_upd, np.float32)
    be = np.asarray(b_edge, np.float32)

    wmats = {
        "wh0r": _bd(Wg0[64:, :64]), "wh0u": _bd(Wg0[64:, 64:]),
        "wh1r": _bd(Wg1[64:, :64]), "wh1u": _bd(Wg1[64:, 64:]),
        "wx0r": _bd(Wg0[:64, :64]), "wx0u": _bd(Wg0[:64, 64:]),
        "wx1r": _bd(Wg1[:64, :64]), "wx1u": _bd(Wg1[:64, 64:]),
        "whc0": _bd(Wu0[64:]), "whc1": _bd(Wu1[64:]),
        "wxc0": _bd(Wu0[:64]), "wxc1": _bd(Wu1[:64]),
        "we": _bd(We),
    }
    wpack = np.concatenate([wmats[n] for n in W_NAMES]
                           + [np.eye(128, dtype=np.float32)], axis=1)
    bpack = np.stack([np.tile(bg[:64], 2), np.tile(bg[64:], 2),
                      np.tile(bu, 2), np.tile(be, 2)], axis=1)
    shared = {
        "lt": np.ascontiguousarray(L.T),
        "wpack": np.ascontiguousarray(wpack),
        "bpack": np.ascontiguousarray(bpack.astype(np.float32)),
    }
    in_maps = []
    for core in range(NCORES):
        xs = x[core * BL:(core + 1) * BL]                    # [BL, N, F]
        m = dict(shared)
        m["xnat"] = np.ascontiguousarray(xs.transpose(1, 0, 2).reshape(N, C))
        m["xt"] = np.ascontiguousarray(xs.transpose(0, 2, 1).reshape(C, N))
        in_maps.append(m)
    return in_maps


def unshard(core_outs):
    """[NCORES][T, C, N] -> [T, B, N, F]"""
    arr = np.stack(core_outs)                                # [8, T, 128, N]
    return np.ascontiguousarray(
        arr.reshape(NCORES, T, BL, F, N)
           .transpose(1, 0, 2, 4, 3)
           .reshape(T, B, N, F).astype(np.float32))


def run(in_maps, **kw):
    nc = _build()
    return run_bass_kernel_spmd(nc, in_maps, list(range(NCORES)), **kw)


def kernel(inputs_edge, L_tilde, W_gate, b_gate, W_upd, b_upd, W_edge, b_edge):
    in_maps = make_in_maps(inputs_edge, L_tilde, W_gate, b_gate,
                           W_upd, b_upd, W_edge, b_edge)
    res = run(in_maps)
    return unshard([res.results[c]["out"] for c in range(NCORES)])


# revision 25
# speedup vs baseline: 15292.0772x; 1.1108x over previous
# Trainium2 Bass kernel for the Chebyshev-GCN GRU decoder (gnn_message_passing).
#
# Problem: B=16, N=2048, F=64, K=2 Chebyshev taps, T=8 decode steps.
#   per step: gates = cheb(L, [x, hx]) @ W_gate; r,u = sigmoid(gates)
#             cy = tanh(cheb(L, [x, r*hx]) @ W_upd); hy = u*hx + (1-u)*cy
#             yt = sigmoid(hy @ W_edge)
#
# Strategy (all math on device; host does only layout transforms + sharding):
#  - Data-parallel over batch: 8 cores x 2 batches each.
#  - x is re-fed every step, so all x-only terms are step-invariant:
#      L@x, x@W*_x, (L@x)@W*_x  ->  computed once on device ("Gconst"/"Cconst").
#    The per-step big matmuls shrink to L@hx and L@(r*hx)  (N x N x 64 each).
#  - Everything lives in "transposed" layout [c, n] with c = b*64 + f (128
#    partitions = 2 batches x 64 features), so the small (feature) matmuls
#    contract over partitions.  The two batches are kept independent in one
#    128-wide matmul by block-diagonal 128x128 weights (built on host).
#  - Big matmul orientation: out[c, n] += sum_m hx_nat[m, c] * L^T[m, n]:
#    stationary = hx in natural layout (16 tiles of [128m, 128c]), moving =
#    L^T (free dim 512 per instruction).  L^T stays fully SBUF-resident
#    (fits because the step-invariant gate consts are stored bf16).
#  - hx/r*hx needed both transposed (elementwise/small-mm) and natural
#    (stationary): regenerated each step with PE-mode transposes (16+16 tiles).
#  - Matmul dtype is a knob: float32r (single-pass fp32, full PE rate at free
#    dim >= 256) vs float32 (exact, 4 cycles/row).  PSUM accumulation is fp32
#    either way.
#
# The kernel() entry point takes FULL unsharded inputs and returns the FULL
# [T, B, N, F] output; it shards/reassembles on host.

import numpy as np
from contextlib import ExitStack

import concourse.bass as bass
import concourse.tile as tile
from concourse import bacc, mybir
from concourse.bass_utils import run_bass_kernel_spmd

F32 = mybir.dt.float32

B, N, F = 16, 2048, 64
T = 8
NCORES = 8
BL = B // NCORES          # batches per core (2)
C = BL * F                # 128 = partition width of transposed tensors
NT = N // 128             # 16 contraction tiles
NBLK = 4                  # n blocks per big matmul
BLK = N // NBLK           # 512 = free dim per matmul instruction
LT_RES = NT               # all 16 L^T row-blocks stay SBUF-resident

# Matmul-operand dtype knob.  float32r = single-pass fp32 matmul (full PE
# rate at free dim >= 256, reduced multiply precision); float32 = exact,
# 4 cycles/row.  walrus requires fp32r operands to be *produced* as fp32r,
# so every tensor feeding a matmul is declared MM_DT end-to-end (same bytes
# as fp32 in memory; numpy side stays float32).
MM_DT = mybir.dt.float32r
BF16 = mybir.dt.bfloat16   # storage dtype of the step-invariant gate consts

W_NAMES = [
    "wh0r", "wh0u", "wh1r", "wh1u",   # gates, hx / L@hx terms (r and u halves)
    "wx0r", "wx0u", "wx1r", "wx1u",   # gates, x / L@x terms (precompute)
    "whc0", "whc1",                   # candidate, r*hx / L@(r*hx) terms
    "wxc0", "wxc1",                   # candidate, x / L@x terms (precompute)
    "we",                             # edge output projection
]
B_NAMES = ["bgr", "bgu", "bcc", "bee"]


def _emit(ctx: ExitStack, tc: tile.TileContext, d):
    """Emit the per-core program.  `d` maps dram tensor name -> AP."""
    nc = tc.nc
    AF = mybir.ActivationFunctionType

    consts = ctx.enter_context(tc.tile_pool(name="consts", bufs=1))
    work = ctx.enter_context(tc.tile_pool(name="work", bufs=2))
    tmp3 = ctx.enter_context(tc.tile_pool(name="tmp", bufs=3))
    big_ps = ctx.enter_context(tc.tile_pool(name="bigps", bufs=2, space="PSUM"))
    small_ps = ctx.enter_context(tc.tile_pool(name="smallps", bufs=4, space="PSUM"))
    tr_ps = ctx.enter_context(tc.tile_pool(name="trps", bufs=2, space="PSUM"))

    # ---- static loads -------------------------------------------------
    # all 13 weights + identity packed into one DMA; biases in another
    wpack = consts.tile([128, 14 * 128], MM_DT, tag="wpack")
    nc.sync.dma_start(wpack[:], d["wpack"][:, :])
    w = {name: wpack[:, i * 128:(i + 1) * 128]
         for i, name in enumerate(W_NAMES)}
    ident = wpack[:, 13 * 128:14 * 128]
    bpack = consts.tile([128, len(B_NAMES)], F32, tag="bpack")
    nc.sync.dma_start(bpack[:], d["bpack"][:, :])
    bias = {name: bpack[:, j:j + 1] for j, name in enumerate(B_NAMES)}

    # stationary buffer: holds x_nat, then alternates hx_nat -> rh_nat per step
    s_sb = consts.tile([128, N], MM_DT, tag="s")
    try:
        nc.sync.dma_start(s_sb[:].rearrange("p (a c) -> p a c", c=128),
                          d["xnat"].rearrange("(a p) c -> p a c", p=128))
    except Exception:
        for mi in range(NT):
            nc.sync.dma_start(s_sb[:, mi * 128:(mi + 1) * 128],
                              d["xnat"][mi * 128:(mi + 1) * 128, :])
    # xt shares its slot with uT (dead after precompute)
    xt_sb = consts.tile([128, N], MM_DT, tag="xt_u")
    nc.sync.dma_start(xt_sb[:], d["xt"][:, :])

    # L^T load is the long pole at kernel start (~14MB): emitted LAST among
    # the static loads (the dynamic-DGE ring issues in order, so anything
    # after it would stall ~40us) and as 256KB chunks in the order the
    # first big matmuls consume them, so PE starts once chunk (0,0) lands.
    lt_sb = consts.tile([128, LT_RES * N], MM_DT, tag="ltsb")
    for blk in range(NBLK):
        for mi in range(LT_RES):
            nc.sync.dma_start(
                lt_sb[:, mi * N + blk * BLK: mi * N + (blk + 1) * BLK],
                d["lt"][mi * 128:(mi + 1) * 128, blk * BLK:(blk + 1) * BLK])

    hxbuf = [consts.tile([128, N], MM_DT, tag=f"hxT{i}", name=f"hxT{i}")
             for i in range(2)]
    rhT = consts.tile([128, N], MM_DT, tag="rhT")
    grc = consts.tile([128, N], BF16, tag="grc")
    guc = consts.tile([128, N], BF16, tag="guc")
    ccc = consts.tile([128, N], BF16, tag="ccc")
    # full-width landing buffer for (L @ stationary)^T; shared by the
    # precompute LxT, phase-A LhT and phase-B LrhT (disjoint lifetimes)
    lxh = consts.tile([128, N], MM_DT, tag="lxh")

    def lt_rhs(mi, blk):
        """[128, BLK] moving-operand slice of L^T for row-block mi, n-block blk."""
        return lt_sb[:, mi * N + blk * BLK: mi * N + (blk + 1) * BLK]

    def big_mm(blk):
        """psum[c, n_blk] = sum_m s_sb[m, c] * L^T[m, n_blk]  (16-tile accum)."""
        ps = big_ps.tile([128, BLK], F32, tag="big")
        for mi in range(NT):
            nc.tensor.matmul(
                ps[:],
                s_sb[:, mi * 128:(mi + 1) * 128],
                lt_rhs(mi, blk),
                start=(mi == 0), stop=(mi == NT - 1))
        return ps

    def small_mm(pairs, const_ap=None):
        """psum = sum_i w_i.T @ rhs_i; then += const_ap in place (DVE)."""
        ps = small_ps.tile([128, BLK], F32, tag="small")
        for i, (wt, rhs) in enumerate(pairs):
            nc.tensor.matmul(ps[:], wt[:], rhs,
                             start=(i == 0), stop=(i == len(pairs) - 1))
        if const_ap is not None:
            nc.vector.tensor_add(ps[:], ps[:], const_ap)
        return ps

    def transpose_to_s(src, blk):
        """PE-transpose 4 [128,128] tiles of src n-block blk into s_sb."""
        pt = tr_ps.tile([128, BLK], MM_DT, tag="tr")
        for j in range(4):
            mi = blk * 4 + j
            nc.tensor.transpose(pt[:, j * 128:(j + 1) * 128],
                                src[:, mi * 128:(mi + 1) * 128], ident[:])
        nc.vector.tensor_copy(s_sb[:, blk * BLK:(blk + 1) * BLK], pt[:])

    def nb(ap, blk):
        return ap[:, blk * BLK:(blk + 1) * BLK]

    # ---- precompute: LxT = (L@x)^T, then the step-invariant gate/cand consts
    for blk in range(NBLK):
        ps = big_mm(blk)                       # s_sb holds x_nat here
        nc.vector.tensor_copy(nb(lxh, blk), ps[:])
    for blk in range(NBLK):
        for wa, wb, bi, dst in (("wx0r", "wx1r", "bgr", grc),
                                ("wx0u", "wx1u", "bgu", guc),
                                ("wxc0", "wxc1", "bcc", ccc)):
            psg = small_mm([(w[wa], nb(xt_sb, blk)), (w[wb], nb(lxh, blk))])
            nc.scalar.activation(nb(dst, blk), psg[:], AF.Identity, bias=bias[bi][:])

    uT = consts.tile([128, N], F32, tag="xt_u")   # reuses xt slot

    def emit_out(t, hyT, blk):
        """yt = sigmoid(W_edge.T @ hy + b_edge) -> DRAM out[t]."""
        ps = small_mm([(w["we"], nb(hyT, blk))])
        ytt = tmp3.tile([128, BLK], F32, tag="tmp")
        nc.scalar.activation(ytt[:], ps[:], AF.Sigmoid, bias=bias["bee"][:])
        nc.sync.dma_start(d["out"][t, :, blk * BLK:(blk + 1) * BLK], ytt[:])

    # ---- step 0 (hx == 0: no big matmuls, r unused) -------------------
    hyT = hxbuf[1]
    for blk in range(NBLK):
        nc.scalar.activation(nb(uT, blk), nb(guc, blk), AF.Sigmoid)
        cyt = work.tile([128, BLK], F32, tag="cyt")
        nc.scalar.activation(cyt[:], nb(ccc, blk), AF.Tanh)
        e = tmp3.tile([128, BLK], F32, tag="tmp")
        nc.vector.tensor_mul(e[:], nb(uT, blk), cyt[:])
        nc.vector.tensor_sub(nb(hyT, blk), cyt[:], e[:])   # hy0 = (1-u)*cy
        emit_out(0, hyT, blk)
        transpose_to_s(hyT, blk)

    # ---- steps 1..T-1 -------------------------------------------------
    for t in range(1, T):
        hxT, hyT = hxbuf[t % 2], hxbuf[(t + 1) % 2]
        # phase A1: Lh = (L@hx)^T for ALL blocks (s_sb must stay hx_nat
        # until every big matmul has read it)
        for blk in range(NBLK):
            ps = big_mm(blk)                   # s_sb holds hx_nat
            nc.vector.tensor_copy(nb(lxh, blk), ps[:])
        # phase A2: r,u; rh = r*hx; transpose rh -> s_sb.
        # Emission order = engine-queue order, so everything on the
        # r-critical path (r matmuls, r const-adds, r sigmoids, rh muls,
        # transposes) is emitted before the u-gate work, which is only
        # needed late in phase B2.
        psrs = [small_mm([(w["wh0r"], nb(hxT, blk)),
                          (w["wh1r"], nb(lxh, blk))],
                         const_ap=nb(grc, blk)) for blk in range(NBLK)]
        psus = [small_mm([(w["wh0u"], nb(hxT, blk)),
                          (w["wh1u"], nb(lxh, blk))]) for blk in range(NBLK)]
        for blk in range(NBLK):
            nc.scalar.activation(nb(rhT, blk), psrs[blk][:], AF.Sigmoid)
        for blk in range(NBLK):
            nc.vector.tensor_mul(nb(rhT, blk), nb(rhT, blk), nb(hxT, blk))
        for blk in range(NBLK):
            transpose_to_s(rhT, blk)
        for blk in range(NBLK):
            nc.vector.tensor_add(psus[blk][:], psus[blk][:], nb(guc, blk))
            nc.scalar.activation(nb(uT, blk), psus[blk][:], AF.Sigmoid)
        # off-critical blend prep (runs while PE does phase B1): the hyT
        # ping-pong buffer is dead from here on (B2 no longer reads hx), so
        # stage W = u*hx directly in it; then uT := (1-u) in place.  The
        # post-tanh chain at each step boundary becomes tanh -> mul -> add.
        for blk in range(NBLK):
            nc.vector.tensor_mul(nb(hyT, blk), nb(uT, blk), nb(hxT, blk))
        for blk in range(NBLK):
            nc.vector.tensor_scalar(nb(uT, blk), nb(uT, blk), -1.0, 1.0,
                                    op0=mybir.AluOpType.mult,
                                    op1=mybir.AluOpType.add)
        # phase B1: Lrh = (L@(r*hx))^T for ALL blocks
        for blk in range(NBLK):
            ps = big_mm(blk)                   # s_sb holds rh_nat
            nc.scalar.copy(nb(lxh, blk), ps[:])
        # phase B2: cy; hy; yt; transpose hy -> s_sb (same split as A2)
        pscs = []
        for blk in range(NBLK):
            pscs.append(small_mm([(w["whc0"], nb(rhT, blk)),
                                  (w["whc1"], nb(lxh, blk))],
                                 const_ap=nb(ccc, blk)))
        for blk in range(NBLK):
            cyt = work.tile([128, BLK], F32, tag="cyt")
            nc.scalar.activation(cyt[:], pscs[blk][:], AF.Tanh)
            pp = tmp3.tile([128, BLK], F32, tag="tmp")
            nc.vector.tensor_mul(pp[:], nb(uT, blk), cyt[:])   # (1-u)*cy
            nc.vector.tensor_add(nb(hyT, blk), nb(hyT, blk), pp[:])
            if t < T - 1:
                transpose_to_s(hyT, blk)
            emit_out(t, hyT, blk)


_BUILT = {}


def _build():
    if "nc" in _BUILT:
        return _BUILT["nc"]
    nc = bacc.Bacc("TRN2", target_bir_lowering=False, debug=False)
    d = {}
    d["lt"] = nc.dram_tensor("lt", [N, N], MM_DT, kind="ExternalInput").ap()
    d["xnat"] = nc.dram_tensor("xnat", [N, C], MM_DT, kind="ExternalInput").ap()
    d["xt"] = nc.dram_tensor("xt", [C, N], MM_DT, kind="ExternalInput").ap()
    d["wpack"] = nc.dram_tensor("wpack", [128, 14 * 128], MM_DT,
                                kind="ExternalInput").ap()
    d["bpack"] = nc.dram_tensor("bpack", [128, len(B_NAMES)], F32,
                                kind="ExternalInput").ap()
    d["out"] = nc.dram_tensor("out", [T, C, N], F32, kind="ExternalOutput").ap()

    with tile.TileContext(nc) as tc, ExitStack() as ctx:
        _emit(ctx, tc, d)
    nc.compile()
    _BUILT["nc"] = nc
    return nc


def _bd(m):
    """[64,64] -> block-diagonal [128,128] (two independent batches)."""
    z = np.zeros((128, 128), np.float32)
    z[:64, :64] = m
    z[64:, 64:] = m
    return z


def make_in_maps(inputs_edge, L_tilde, W_gate, b_gate, W_upd, b_upd,
                 W_edge, b_edge):
    """Host-side layout transforms + per-core sharding (no math)."""
    x = np.asarray(inputs_edge, np.float32)
    L = np.asarray(L_tilde, np.float32)
    Wg0, Wg1 = np.asarray(W_gate[0], np.float32), np.asarray(W_gate[1], np.float32)
    Wu0, Wu1 = np.asarray(W_upd[0], np.float32), np.asarray(W_upd[1], np.float32)
    We = np.asarray(W_edge, np.float32)
    bg = np.asarray(b_gate, np.float32)
    bu = np.asarray(b_upd, np.float32)
    be = np.asarray(b_edge, np.float32)

    wmats = {
        "wh0r": _bd(Wg0[64:, :64]), "wh0u": _bd(Wg0[64:, 64:]),
        "wh1r": _bd(Wg1[64:, :64]), "wh1u": _bd(Wg1[64:, 64:]),
        "wx0r": _bd(Wg0[:64, :64]), "wx0u": _bd(Wg0[:64, 64:]),
        "wx1r": _bd(Wg1[:64, :64]), "wx1u": _bd(Wg1[:64, 64:]),
        "whc0": _bd(Wu0[64:]), "whc1": _bd(Wu1[64:]),
        "wxc0": _bd(Wu0[:64]), "wxc1": _bd(Wu1[:64]),
        "we": _bd(We),
    }
    wpack = np.concatenate([wmats[n] for n in W_NAMES]
                           + [np.eye(128, dtype=np.float32)], axis=1)
    bpack = np.stack([np.tile(bg[:64], 2), np.tile(bg[64:], 2),
                      np.tile(bu, 2), np.tile(be, 2)], axis=1)
    shared = {
        "lt": np.ascontiguousarray(L.T),
        "wpack": np.ascontiguousarray(wpack),
        "bpack": np.ascontiguousarray(bpack.astype(np.float32)),
    }
    in_maps = []
    for core in range(NCORES):
        xs = x[core * BL:(core + 1) * BL]                    # [BL, N, F]
        m = dict(shared)
        m["xnat"] = np.ascontiguousarray(xs.transpose(1, 0, 2).reshape(N, C))
        m["xt"] = np.ascontiguousarray(xs.transpose(0, 2, 1).reshape(C, N))
        in_maps.append(m)
    return in_maps


def unshard(core_outs):
    """[NCORES][T, C, N] -> [T, B, N, F]"""
    arr = np.stack(core_outs)                                # [8, T, 128, N]
    return np.ascontiguousarray(
        arr.reshape(NCORES, T, BL, F, N)
           .transpose(1, 0, 2, 4, 3)
           .reshape(T, B, N, F).astype(np.float32))


def run(in_maps, **kw):
    nc = _build()
    return run_bass_kernel_spmd(nc, in_maps, list(range(NCORES)), **kw)


def kernel(inputs_edge, L_tilde, W_gate, b_gate, W_upd, b_upd, W_edge, b_edge):
    in_maps = make_in_maps(inputs_edge, L_tilde, W_gate, b_gate,
                           W_upd, b_upd, W_edge, b_edge)
    res = run(in_maps)
    return unshard([res.results[c]["out"] for c in range(NCORES)])
